# revision 1
# baseline (speedup 1.0000x reference)
"""GNN message-passing encoder (PyG GraphConv x4) on 8 TRN2 NeuronCores.

Strategy (graph/data parallel, per sharding hint):
  - Nodes are sharded by destination range: core c owns dst nodes
    [c*6250, (c+1)*6250).  Each core processes the ~100K edges whose dst it
    owns.
  - Per layer the aggregation `segment_sum(x[src], dst)` runs as:
      gather rows of a replicated DRAM table via dma_gather (per-edge
      descriptors), then segment-sum on the PE with per-window one-hot
      indicator matmuls accumulated in PSUM (49 windows of 128 dst nodes).
    Indicators are built on the DVE with a broadcast is_equal against an
    iota tile.  Degree counts ride along as a ones-column of the L1 table.
  - Layer 1 aggregates x (64ch) directly; layer 2 aggregates h1 @ W2_rel
    (128ch, transform-before-aggregate); the two output heads share one
    aggregation of [h2@Wmu_rel | h2@Wls_rel] (32ch).
  - Between layers the per-core transformed tables are AllGathered so every
    core can gather any source row.  Weights are replicated.

dma_gather indices are int16, so each 50176-row table is split in two
halves with rebased indices; every (window, half) region is padded to a
fixed capacity so the single SPMD program fits all cores.
"""

import sys

sys.path.insert(0, "/opt/trn_rl_repo")

import numpy as np

import concourse.bass as bass
import concourse.bacc as bacc
import concourse.mybir as mybir
from concourse import tile
from concourse.bass import AP

# ---------------------------------------------------------------- constants
import os as _os

_SMALL = bool(int(_os.environ.get("GNN_SMALL", "0")))

IN_CH = 64
OC = 16
D1 = 256
D2 = 128
N_CORES = 8

if _SMALL:
    N_NODES = 4096
    N_EDGES = 16384
    LOCAL = 512
    NW = 4
    HALF_A = 2048
    CAP_A = 384
    CAP_B = 384
else:
    N_NODES = 50000
    N_EDGES = 800000
    LOCAL = N_NODES // N_CORES      # 6250
    NW = 49                         # dst windows of 128 per core
    HALF_A = 25088                  # balanced halves, both int16-addressable
    CAP_A = 1152                    # slots per (window, half-A); 9 chunks
    CAP_B = 1152                    # slots per (window, half-B); 9 chunks

LOCAL_PAD = NW * 128
RT = N_CORES * LOCAL_PAD
NCH_A = CAP_A // 128
NCH_B = CAP_B // 128

F32 = mybir.dt.float32
I16 = mybir.dt.int16

# weights blob layout: name -> (rows, cols)
_WB_SPEC = [
    ("iota", 128, 128),
    ("ident", 128, 128),
    ("Wstack1", 65, D1),    # [W1_rel; b1]
    ("W1_root", 64, D1),
    ("W2_rel_h0", 128, D2),
    ("W2_rel_h1", 128, D2),
    ("W2_root_h0", 128, D2),
    ("W2_root_h1", 128, D2),
    ("b2row", 1, D2),
    ("Wheads", D2, 2 * OC),  # [Wmu_rel | Wls_rel]
    ("Wroots", D2, 2 * OC),  # [Wmu_root | Wls_root]
    ("bheads", 1, 2 * OC),   # [bmu | bls]
]
_WB_OFF = {}
_off = 0
for _n, _r, _c in _WB_SPEC:
    _WB_OFF[_n] = (_off, _r, _c)
    _off += _r * _c
WB_LEN = _off

_NC_CACHE = None


def _bcast3(ap2d: AP, mid: int, inner: int, mode: str) -> AP:
    """[128, X] -> [128, mid, inner] with a 0-stride broadcast dim."""
    if mode == "inner0":   # [128, mid] -> [128, mid, inner], inner step 0
        return AP(ap2d.tensor, ap2d.offset, [ap2d.ap[0], ap2d.ap[1], [0, inner]])
    if mode == "mid0":     # [128, inner] -> [128, mid, inner], mid step 0
        return AP(ap2d.tensor, ap2d.offset, [ap2d.ap[0], [0, mid], ap2d.ap[1]])
    raise ValueError(mode)


def _build_nc():
    import os
    phases = int(os.environ.get("GNN_PHASES", "5"))
    nc = bacc.Bacc(None, num_swdge_queues=4, dynamic_dma_scratch_size=32768)

    # ---- IO
    x_pad = nc.dram_tensor("x_pad", [RT, 128], F32, kind="ExternalInput")
    x_tr_d = nc.dram_tensor("x_tr", [64, LOCAL_PAD], F32, kind="ExternalInput")
    gidxA = nc.dram_tensor("gidxA", [128, NW * CAP_A // 16], I16, kind="ExternalInput")
    gidxB = nc.dram_tensor("gidxB", [128, NW * CAP_B // 16], I16, kind="ExternalInput")
    drelA = nc.dram_tensor("drelA", [128, NW * NCH_A], F32, kind="ExternalInput")
    drelB = nc.dram_tensor("drelB", [128, NW * NCH_B], F32, kind="ExternalInput")
    wb = nc.dram_tensor("wb", [WB_LEN], F32, kind="ExternalInput")
    out_d = nc.dram_tensor("out", [LOCAL_PAD, 2 * OC], F32, kind="ExternalOutput")

    # ---- internal DRAM
    m2loc = nc.dram_tensor("m2loc", [LOCAL_PAD, D2], F32)
    tloc = nc.dram_tensor("tloc", [LOCAL_PAD, 64], F32)
    m2_full = nc.dram_tensor("m2_full", [RT, D2], F32, addr_space="Shared")
    t_full = nc.dram_tensor("t_full", [RT, 64], F32, addr_space="Shared")

    RG = [list(range(N_CORES))]

    with tile.TileContext(nc) as tc:
        with (
            tc.tile_pool(name="cst", bufs=1) as cst,
            tc.tile_pool(name="big", bufs=1) as big,
            tc.tile_pool(name="slab", bufs=2) as slabp,
            tc.tile_pool(name="ind", bufs=2) as indp,
            tc.tile_pool(name="sm", bufs=3) as sm,
            tc.tile_pool(name="stg", bufs=2) as stgp,
            tc.tile_pool(name="pw", bufs=3, space="PSUM") as pwp,
            tc.tile_pool(name="pt", bufs=1, space="PSUM") as ptp,
            tc.tile_pool(name="ph", bufs=2, space="PSUM") as php,
        ):
            # ---------------- constants / weights
            wt = {}
            for name, r, c in _WB_SPEC:
                t = cst.tile([r, c], F32, tag=f"w_{name}")
                off = _WB_OFF[name][0]
                nc.sync.dma_start(
                    t[:], wb[off:off + r * c].rearrange("(r c) -> r c", c=c)
                )
                wt[name] = t
            onesrow = cst.tile([1, 128], F32, tag="onesrow")
            nc.vector.memset(onesrow[:], 1.0)

            gixA = cst.tile([128, NW * CAP_A // 16], I16, tag="gixA")
            gixB = cst.tile([128, NW * CAP_B // 16], I16, tag="gixB")
            dstA = cst.tile([128, NW * NCH_A], F32, tag="dstA")
            dstB = cst.tile([128, NW * NCH_B], F32, tag="dstB")
            nc.sync.dma_start(gixA[:], gidxA[:])
            nc.sync.dma_start(gixB[:], gidxB[:])
            nc.sync.dma_start(dstA[:], drelA[:])
            nc.sync.dma_start(dstB[:], drelB[:])

            x_tr = big.tile([64, LOCAL_PAD], F32, tag="x_tr")
            nc.sync.dma_start(x_tr[:], x_tr_d[:])

            h1T0 = big.tile([128, LOCAL_PAD], F32, tag="h1T0")
            h1T1 = big.tile([128, LOCAL_PAD], F32, tag="h1T1")
            h2T = big.tile([128, LOCAL_PAD], F32, tag="h2T")
            dinv_all = big.tile([128, NW], F32, tag="dinv")

            iota_t = wt["iota"]
            ident = wt["ident"]

            # SWDGE queue discipline: Tile round-robins SWDGE completions over
            # 8 DMASW sem lanes in *scheduled* POOL order, and each lane must
            # stay on one queue.  So gathers are (a) chained with no-sync deps
            # to pin their POOL order to emission order and (b) issued on
            # queues rotating with period 4 (8 lanes % 4 queues -> each lane
            # sees exactly one queue).
            gchain = [None]
            gq = [0]

            def gather_chained(out_ap, table_ap, idx_ap, n, elem):
                ins = nc.gpsimd.dma_gather(
                    out_ap, table_ap, idx_ap, n, n, elem, queue_num=gq[0]
                )
                gq[0] = (gq[0] + 1) % 4
                if gchain[0] is not None:
                    bass._add_dep_helper(
                        ins.ins, gchain[0].ins, sync=False,
                        reason="pin SWDGE pool order for queue/lane pairing",
                    )
                gchain[0] = ins
                return ins

            def agg_window(w, tableA_ap, tableB_ap, elem, dcols, queue):
                """Gather + indicator matmuls for window w.
                Returns the accumulated PSUM tile [128, dcols].
                dma_gather is capped at 1024 indices per call (64 descriptors
                per DMA engine), so the A region is fetched in two calls."""
                slabA = slabp.tile([128, NCH_A, elem], F32, tag="slabA")
                a0 = w * (CAP_A // 16)
                for lo in range(0, CAP_A, 1024):
                    n = min(1024, CAP_A - lo)
                    gather_chained(
                        slabA[:, lo // 128:(lo + n) // 128, :], tableA_ap,
                        gixA[:, a0 + lo // 16:a0 + (lo + n) // 16], n, elem,
                    )
                slabB = slabp.tile([128, NCH_B, elem], F32, tag="slabB")
                for lo in range(0, CAP_B, 1024):
                    n = min(1024, CAP_B - lo)
                    gather_chained(
                        slabB[:, lo // 128:(lo + n) // 128, :], tableB_ap,
                        gixB[:, w * (CAP_B // 16) + lo // 16:
                             w * (CAP_B // 16) + (lo + n) // 16], n, elem,
                    )
                indA = indp.tile([128, NCH_A, 128], F32, tag="indA")
                nc.vector.tensor_tensor(
                    indA[:],
                    _bcast3(dstA[:, w * NCH_A:(w + 1) * NCH_A], NCH_A, 128, "inner0"),
                    _bcast3(iota_t[:], NCH_A, 128, "mid0"),
                    mybir.AluOpType.is_equal,
                )
                indB = indp.tile([128, NCH_B, 128], F32, tag="indB")
                nc.vector.tensor_tensor(
                    indB[:],
                    _bcast3(dstB[:, w * NCH_B:(w + 1) * NCH_B], NCH_B, 128, "inner0"),
                    _bcast3(iota_t[:], NCH_B, 128, "mid0"),
                    mybir.AluOpType.is_equal,
                )
                pw = pwp.tile([128, dcols], F32, tag="pw")
                nch = NCH_A + NCH_B
                k = 0
                for c in range(NCH_A):
                    nc.tensor.matmul(pw[:], indA[:, c, :], slabA[:, c, 0:dcols],
                                     start=(k == 0), stop=(k == nch - 1))
                    k += 1
                for c in range(NCH_B):
                    nc.tensor.matmul(pw[:], indB[:, c, :], slabB[:, c, 0:dcols],
                                     start=(k == 0), stop=(k == nch - 1))
                    k += 1
                return pw

            # ======================= L1 =======================
            for w in range(NW):
                ws = slice(w * 128, (w + 1) * 128)
                pw = agg_window(w, x_pad[0:HALF_A, :], x_pad[HALF_A:RT, :],
                                128, 65, w % 2)
                # deg -> deg_inv: recip(max(deg,1)) * min(deg,1)
                mx = sm.tile([128, 1], F32, tag="mx")
                nc.vector.tensor_scalar_max(mx[:], pw[:, 64:65], 1.0)
                rc = sm.tile([128, 1], F32, tag="rc")
                nc.vector.reciprocal(rc[:], mx[:])
                mn = sm.tile([128, 1], F32, tag="mn")
                nc.vector.tensor_scalar_min(mn[:], pw[:, 64:65], 1.0)
                dinv = sm.tile([128, 1], F32, tag="dinv_w")
                nc.vector.tensor_tensor(dinv[:], rc[:], mn[:], mybir.AluOpType.mult)
                nc.vector.tensor_copy(dinv_all[:, w:w + 1], dinv[:])
                # agg_nm = pw[:, :64] * dinv
                agg_nm = sm.tile([128, 64], F32, tag="agg_nm")
                nc.vector.tensor_scalar_mul(agg_nm[:], pw[:, 0:64], dinv[:])
                # aggT = [transpose(agg_nm); ones]
                ptr = ptp.tile([64, 128], F32, tag="ptr")
                nc.tensor.transpose(ptr[:], agg_nm[:], ident[:])
                aggT = sm.tile([65, 128], F32, tag="aggT")
                nc.vector.tensor_copy(aggT[0:64, :], ptr[:])
                nc.vector.memset(aggT[64:65, :], 1.0)
                # h1T halves
                for half, h1t in ((0, h1T0), (1, h1T1)):
                    hs = slice(half * 128, (half + 1) * 128)
                    ph = php.tile([128, 128], F32, tag="ph")
                    nc.tensor.matmul(ph[:], wt["Wstack1"][:, hs], aggT[:],
                                     start=True, stop=False)
                    nc.tensor.matmul(ph[:], wt["W1_root"][:, hs], x_tr[:, ws],
                                     start=False, stop=True)
                    nc.scalar.activation(
                        h1t[:, ws], ph[:], mybir.ActivationFunctionType.Relu
                    )
                # m2 for this window rides inside the L1 loop so its PE and
                # DMA work overlaps the gather stream
                if phases >= 2:
                    pm = php.tile([128, D2], F32, tag="ph")
                    nc.tensor.matmul(pm[:], h1T0[:, ws], wt["W2_rel_h0"][:],
                                     start=True, stop=False)
                    nc.tensor.matmul(pm[:], h1T1[:, ws], wt["W2_rel_h1"][:],
                                     start=False, stop=True)
                    stg = stgp.tile([128, D2], F32, tag="m2stg")
                    nc.vector.tensor_copy(stg[:], pm[:])
                    nc.sync.dma_start(m2loc[w * 128:(w + 1) * 128, :], stg[:])

            if phases >= 2:
                nc.gpsimd.collective_compute(
                    "AllGather", mybir.AluOpType.bypass, replica_groups=RG,
                    ins=[m2loc[:]], outs=[m2_full[:]],
                )
            # ======================= L2 + h2 =======================
            for w in range(NW if phases >= 3 else 0):
                ws = slice(w * 128, (w + 1) * 128)
                pw = agg_window(w, m2_full[0:HALF_A, :], m2_full[HALF_A:RT, :],
                                D2, D2, w % 2)
                agg2 = sm.tile([128, D2], F32, tag="agg2")
                nc.vector.tensor_scalar_mul(agg2[:], pw[:], dinv_all[:, w:w + 1])
                # transpose must be its own PSUM group (mixing transpose-mode
                # into an accumulation group poisons PSUM on HW)
                ptr2 = ptp.tile([128, 128], F32, tag="ptr2")
                nc.tensor.transpose(ptr2[:], agg2[:], ident[:])
                ph2 = php.tile([128, 128], F32, tag="ph")
                nc.tensor.matmul(ph2[:], wt["W2_root_h0"][:], h1T0[:, ws],
                                 start=True, stop=False)
                nc.tensor.matmul(ph2[:], wt["W2_root_h1"][:], h1T1[:, ws],
                                 start=False, stop=False)
                nc.tensor.matmul(ph2[:], wt["b2row"][:], onesrow[:],
                                 start=False, stop=True)
                tr2 = sm.tile([128, 128], F32, tag="tr2")
                nc.vector.tensor_copy(tr2[:], ptr2[:])
                hsum = sm.tile([128, 128], F32, tag="hsum")
                nc.vector.tensor_tensor(hsum[:], tr2[:], ph2[:],
                                        mybir.AluOpType.add)
                nc.scalar.activation(
                    h2T[:, ws], hsum[:], mybir.ActivationFunctionType.Relu
                )
                if phases >= 4:
                    pt = php.tile([128, 2 * OC], F32, tag="ph")
                    nc.tensor.matmul(pt[:], h2T[:, ws], wt["Wheads"][:],
                                     start=True, stop=True)
                    stg = stgp.tile([128, 64], F32, tag="tstg")
                    nc.vector.tensor_copy(stg[:, 0:2 * OC], pt[:])
                    nc.vector.memset(stg[:, 2 * OC:64], 0.0)
                    nc.sync.dma_start(tloc[w * 128:(w + 1) * 128, :], stg[:])
            if phases >= 4:
                nc.gpsimd.collective_compute(
                    "AllGather", mybir.AluOpType.bypass, replica_groups=RG,
                    ins=[tloc[:]], outs=[t_full[:]],
                )

            # ======================= heads =======================
            for w in range(NW if phases >= 5 else 0):
                ws = slice(w * 128, (w + 1) * 128)
                pw = agg_window(w, t_full[0:HALF_A, :], t_full[HALF_A:RT, :],
                                64, 2 * OC, w % 2)
                pf = php.tile([128, 2 * OC], F32, tag="ph")
                nc.tensor.matmul(pf[:], h2T[:, ws], wt["Wroots"][:],
                                 start=True, stop=False)
                nc.tensor.matmul(pf[:], onesrow[:], wt["bheads"][:],
                                 start=False, stop=True)
                aggh = sm.tile([128, 2 * OC], F32, tag="aggh")
                nc.vector.tensor_scalar_mul(aggh[:], pw[:], dinv_all[:, w:w + 1])
                ot = stgp.tile([128, 2 * OC], F32, tag="ot")
                nc.vector.tensor_tensor(ot[:], aggh[:], pf[:], mybir.AluOpType.add)
                nc.sync.dma_start(out_d[w * 128:(w + 1) * 128, :], ot[:])
            if phases < 5:
                dbg_src = {1: h1T0, 2: h1T0, 3: h2T, 4: h2T}[phases]
                for w in range(NW):
                    nc.sync.dma_start(
                        out_d[w * 128:(w + 1) * 128, :],
                        dbg_src[0:128, w * 128:w * 128 + 2 * OC].rearrange(
                            "p d -> d p"
                        ) if False else dbg_src[:, w * 128:(w + 1) * 128][:, 0:2 * OC],
                    )

    nc.compile()
    return nc


def get_nc():
    global _NC_CACHE
    if _NC_CACHE is None:
        _NC_CACHE = _build_nc()
    return _NC_CACHE


# ---------------------------------------------------------------- host prep

def _wrap_idx16(vals: np.ndarray, nslots: int) -> np.ndarray:
    """Slot-ordered int16 values -> [128, nslots/16] wrapped+tiled layout."""
    a = vals.astype(np.int16).reshape(nslots // 16, 16).T  # [16, W]
    return np.tile(a, (8, 1))


def _prep_core(src_row, dst_local, core_mask):
    """Build gidxA/B, drelA/B arrays for one core."""
    row = src_row[core_mask]
    dl = dst_local[core_mask]
    win = dl >> 7
    rel = (dl & 127).astype(np.float32)

    out = {}
    for half, cap, nch in ((0, CAP_A, NCH_A), (1, CAP_B, NCH_B)):
        sel = (row < HALF_A) if half == 0 else (row >= HALF_A)
        r = row[sel] - (0 if half == 0 else HALF_A)
        wv = win[sel]
        rv = rel[sel]
        order = np.argsort(wv, kind="stable")
        r, wv, rv = r[order], wv[order], rv[order]
        counts = np.bincount(wv, minlength=NW)
        if counts.max() > cap:
            raise RuntimeError(f"window overflow: {counts.max()} > {cap}")
        starts = np.zeros(NW, np.int64)
        starts[1:] = np.cumsum(counts)[:-1]
        pos = np.arange(len(wv)) - np.repeat(starts, counts)
        slot = wv * cap + pos
        nslots = NW * cap
        gvals = np.zeros(nslots, np.int64)
        gvals[slot] = r
        dvals = np.full(nslots, -1.0, np.float32)
        dvals[slot] = rv
        gname = "gidxA" if half == 0 else "gidxB"
        dname = "drelA" if half == 0 else "drelB"
        out[gname] = _wrap_idx16(gvals, nslots)
        out[dname] = dvals.reshape(nslots // 128, 128).T.copy()
    return out


def _pack_weights(i):
    wb = np.zeros(WB_LEN, np.float32)

    def put(name, arr):
        off, r, c = _WB_OFF[name]
        wb[off:off + r * c] = np.asarray(arr, np.float32).reshape(r * c)

    put("iota", np.tile(np.arange(128, dtype=np.float32), (128, 1)))
    put("ident", np.eye(128, dtype=np.float32))
    put("Wstack1", np.concatenate([i["W1_rel"], i["b1"][None, :]], 0))
    put("W1_root", i["W1_root"])
    put("W2_rel_h0", i["W2_rel"][0:128])
    put("W2_rel_h1", i["W2_rel"][128:256])
    put("W2_root_h0", i["W2_root"][0:128])
    put("W2_root_h1", i["W2_root"][128:256])
    put("b2row", i["b2"][None, :])
    put("Wheads", np.concatenate([i["Wmu_rel"], i["Wls_rel"]], 1))
    put("Wroots", np.concatenate([i["Wmu_root"], i["Wls_root"]], 1))
    put("bheads", np.concatenate([i["bmu"], i["bls"]])[None, :])
    return wb


def kernel(**inputs):
    x = np.asarray(inputs["x"], np.float32)
    ei = np.asarray(inputs["edge_index"])
    src = ei[0].astype(np.int64)
    dst = ei[1].astype(np.int64)

    owner = dst // LOCAL
    dst_local = dst - owner * LOCAL
    src_row = (src // LOCAL) * LOCAL_PAD + (src % LOCAL)

    x_pad = np.zeros((RT, 128), np.float32)
    for c in range(N_CORES):
        x_pad[c * LOCAL_PAD:c * LOCAL_PAD + LOCAL, 0:64] = x[c * LOCAL:(c + 1) * LOCAL]
    x_pad[:, 64] = 1.0

    wb = _pack_weights({k: np.asarray(v, np.float32) for k, v in inputs.items()
                        if k not in ("x", "edge_index")})

    in_maps = []
    for c in range(N_CORES):
        m = _prep_core(src_row, dst_local, owner == c)
        x_tr = np.zeros((64, LOCAL_PAD), np.float32)
        x_tr[:, :LOCAL] = x[c * LOCAL:(c + 1) * LOCAL].T
        m["x_pad"] = x_pad
        m["x_tr"] = x_tr
        m["wb"] = wb
        in_maps.append(m)

    from concourse.bass_utils import run_bass_kernel_spmd

    nc = get_nc()
    res = run_bass_kernel_spmd(nc, in_maps, list(range(N_CORES)))

    mu = np.zeros((N_NODES, OC), np.float32)
    ls = np.zeros((N_NODES, OC), np.float32)
    for c in range(N_CORES):
        o = res.results[c]["out"][:LOCAL]
        mu[c * LOCAL:(c + 1) * LOCAL] = o[:, :OC]
        ls[c * LOCAL:(c + 1) * LOCAL] = o[:, OC:]
    return (mu, ls)


if __name__ == "__main__":
    # quick self-test with random data
    rng = np.random.default_rng(0)
    ins = {
        "x": rng.standard_normal((N_NODES, IN_CH)).astype(np.float32),
        "edge_index": rng.integers(0, N_NODES, (2, N_EDGES)),
        "W1_rel": rng.standard_normal((IN_CH, D1)).astype(np.float32) * 0.1,
        "b1": np.zeros(D1, np.float32),
        "W1_root": rng.standard_normal((IN_CH, D1)).astype(np.float32) * 0.1,
        "W2_rel": rng.standard_normal((D1, D2)).astype(np.float32) * 0.1,
        "b2": np.zeros(D2, np.float32),
        "W2_root": rng.standard_normal((D1, D2)).astype(np.float32) * 0.1,
        "Wmu_rel": rng.standard_normal((D2, OC)).astype(np.float32) * 0.1,
        "bmu": np.zeros(OC, np.float32),
        "Wmu_root": rng.standard_normal((D2, OC)).astype(np.float32) * 0.1,
        "Wls_rel": rng.standard_normal((D2, OC)).astype(np.float32) * 0.1,
        "bls": np.zeros(OC, np.float32),
        "Wls_root": rng.standard_normal((D2, OC)).astype(np.float32) * 0.1,
    }
    mu, ls = kernel(**ins)
    print("kernel ran:", mu.shape, ls.shape, mu[:2, :4])



# revision 2
# speedup vs baseline: 1.0167x; 1.0167x over previous
"""GNN message-passing encoder (PyG GraphConv x4) on 8 TRN2 NeuronCores, v2.

Strategy (graph/data parallel): nodes sharded by dst range; per layer the
aggregation segment_sum(x[src], dst) runs as dma_gather of a replicated DRAM
table + per-window indicator matmuls accumulated in PSUM.

v2 design notes:
  - bf16 tables + bf16 indicator matmuls (fp32 PSUM accumulate).
  - indicators (ind[s,d] = dinv[dst_s] * (drel_s==d)) are built on the DVE
    (is_equal vs iota, then mult by the per-slot deg_inv), both halves in one
    [128, 18, 128] tile -> 2 Vector ops per window. The Vector queue carries
    ONLY these ops (all copies ride Scalar), so it never serializes the
    window pipeline. (DMA-ing host-precomputed indicators was tried: the
    640KB/window stream saturated the DMA engines and slowed the gathers.)
  - deg_inv is folded into the indicator values, so the aggregation comes out
    pre-normalized and TRANSPOSED ([ch, dst]) via slab-stationary matmuls.
  - one shared table row layout for all 3 phases (core-major), so one gather
    index set serves x, m2 and h2.
  - gathers: CAP=1152 per (window,half), calls of [640,512] per half with the
    order alternating by window parity -> all 4 SWDGE queues carry equal
    descriptor load (>1024 per call hangs the ucode; unbalanced queues halve
    the drain rate). Slots sorted by table row within each region.
  - all PSUM->SBUF copies ride the Scalar engine (activation Copy), keeping
    the Vector queue out of the dependency chain entirely.
  - one serial AllGather per phase boundary: chunked in-loop AGs were tried
    (7x1.8MB and 3x5MB variants) and LOST - collective DMA contends with the
    gather descriptor stream for the 16 DMA engines, slowing windows ~10%
    and running the AGs themselves at 35-87GB/s vs ~200GB/s uncontended.
  - heads aggregate h2 directly (transform after aggregation), so phase 3
    shares the h2 table with the root term.
"""

import sys

sys.path.insert(0, "/opt/trn_rl_repo")

import numpy as np
import ml_dtypes

import concourse.bass as bass
import concourse.bacc as bacc
import concourse.mybir as mybir
from concourse import tile
from concourse.bass import AP

# ---------------------------------------------------------------- constants
IN_CH = 64
OC = 16
D1 = 256
D2 = 128
N_CORES = 8

N_NODES = 50000
N_EDGES = 800000
LOCAL = N_NODES // N_CORES          # 6250
NW = 49                             # dst windows of 128 per core
LOCAL_PAD = NW * 128                # 6272
RT = N_CORES * LOCAL_PAD            # 50176
HALF_A = 25088                      # rows [0,HALF_A) use gidxA (int16 range)
CAP = 1152                          # slots per (window, half); max real fill 1137
NCH = CAP // 128                    # 9 slab chunks per half
SPLITS = ([640, 512], [512, 640])   # per-window call sizes, alternating so all
                                    # 4 SWDGE queues carry equal load

F32 = mybir.dt.float32
BF16 = mybir.dt.bfloat16
I16 = mybir.dt.int16

BF = ml_dtypes.bfloat16

# weights blob layout: name -> (rows, cols), all bf16
_WB_SPEC = [
    ("iota", 128, 128),
    ("ident", 128, 128),
    ("W1_rel", 64, D1),
    ("W1_root", 64, D1),
    ("b1", 1, D1),
    ("W2_rel_h0", 128, D2),
    ("W2_rel_h1", 128, D2),
    ("W2_root_h0", 128, D2),
    ("W2_root_h1", 128, D2),
    ("b2", 1, D2),
    ("Wheads", D2, 2 * OC),
    ("Wroots", D2, 2 * OC),
    ("bheads", 1, 2 * OC),
]
_WB_OFF = {}
_off = 0
for _n, _r, _c in _WB_SPEC:
    _WB_OFF[_n] = (_off, _r, _c)
    _off += _r * _c
WB_LEN = _off

_NC_CACHE = None


def _bcast3(ap2d: AP, mid: int, inner: int, mode: str) -> AP:
    """[128, X] -> [128, mid, inner] with a 0-stride broadcast dim."""
    if mode == "inner0":   # [128, mid] -> [128, mid, inner], inner step 0
        return AP(ap2d.tensor, ap2d.offset, [ap2d.ap[0], ap2d.ap[1], [0, inner]])
    if mode == "mid0":     # [128, inner] -> [128, mid, inner], mid step 0
        return AP(ap2d.tensor, ap2d.offset, [ap2d.ap[0], [0, mid], ap2d.ap[1]])
    raise ValueError(mode)


def _build_nc():
    import os
    phases = int(os.environ.get("GNN2_PHASES", "3"))
    nc = bacc.Bacc(None, num_swdge_queues=4, dynamic_dma_scratch_size=32768)

    # ---- IO
    x_pad = nc.dram_tensor("x_pad", [RT, 128], BF16, kind="ExternalInput")
    x_tr_d = nc.dram_tensor("x_tr", [64, LOCAL_PAD], BF16, kind="ExternalInput")
    gidxA = nc.dram_tensor("gidxA", [128, NW * CAP // 16], I16, kind="ExternalInput")
    gidxB = nc.dram_tensor("gidxB", [128, NW * CAP // 16], I16, kind="ExternalInput")
    drel_d = nc.dram_tensor("drel", [128, NW * 2 * NCH], BF16, kind="ExternalInput")
    wval_d = nc.dram_tensor("wval", [128, NW * 2 * NCH], BF16, kind="ExternalInput")
    wb = nc.dram_tensor("wb", [WB_LEN], BF16, kind="ExternalInput")
    out_d = nc.dram_tensor("out", [2 * OC, LOCAL_PAD], F32, kind="ExternalOutput")

    # ---- internal DRAM
    m2loc = nc.dram_tensor("m2loc", [LOCAL_PAD, D2], BF16)
    h2loc = nc.dram_tensor("h2loc", [LOCAL_PAD, D2], BF16)
    m2_full = nc.dram_tensor("m2_full", [RT, D2], BF16, addr_space="Shared")
    h2_full = nc.dram_tensor("h2_full", [RT, D2], BF16, addr_space="Shared")

    RG = [list(range(N_CORES))]

    with tile.TileContext(nc) as tc:
        with (
            tc.tile_pool(name="cst", bufs=1) as cst,
            tc.tile_pool(name="big", bufs=1) as big,
            tc.tile_pool(name="slab", bufs=3) as slabp,
            tc.tile_pool(name="ind", bufs=3) as indp,
            tc.tile_pool(name="sm", bufs=3) as sm,
            tc.tile_pool(name="stg", bufs=2) as stgp,
            tc.tile_pool(name="pagg", bufs=2, space="PSUM") as paggp,
            tc.tile_pool(name="ph", bufs=3, space="PSUM") as php,
            tc.tile_pool(name="ptr", bufs=2, space="PSUM") as ptrp,
        ):
            # ---------------- constants / weights
            wt = {}
            for name, r, c in _WB_SPEC:
                t = cst.tile([r, c], BF16, tag=f"w_{name}")
                off = _WB_OFF[name][0]
                nc.sync.dma_start(
                    t[:], wb[off:off + r * c].rearrange("(r c) -> r c", c=c)
                )
                wt[name] = t
            onesrow = cst.tile([1, 128], BF16, tag="onesrow")
            nc.vector.memset(onesrow[:], 1.0)

            gixA = cst.tile([128, NW * CAP // 16], I16, tag="gixA")
            gixB = cst.tile([128, NW * CAP // 16], I16, tag="gixB")
            drt = cst.tile([128, NW * 2 * NCH], BF16, tag="drt")
            wvt = cst.tile([128, NW * 2 * NCH], BF16, tag="wvt")
            nc.sync.dma_start(gixA[:], gidxA[:])
            nc.sync.dma_start(gixB[:], gidxB[:])
            nc.sync.dma_start(drt[:], drel_d[:])
            nc.sync.dma_start(wvt[:], wval_d[:])

            x_tr = big.tile([64, LOCAL_PAD], BF16, tag="x_tr")
            nc.sync.dma_start(x_tr[:], x_tr_d[:])

            h1T0 = big.tile([128, LOCAL_PAD], BF16, tag="h1T0")
            h1T1 = big.tile([128, LOCAL_PAD], BF16, tag="h1T1")
            h2T = big.tile([128, LOCAL_PAD], BF16, tag="h2T")

            ident = wt["ident"]
            COPY = mybir.ActivationFunctionType.Copy
            RELU = mybir.ActivationFunctionType.Relu

            # SWDGE queue discipline: gathers chained with no-sync deps to pin
            # their POOL order to emission order; queues rotate with period 4
            # (8 DMASW sem lanes % 4 queues -> each lane sees one queue).
            gchain = [None]
            gq = [0]

            def gather_chained(out_ap, table_ap, idx_ap, n):
                ins = nc.gpsimd.dma_gather(
                    out_ap, table_ap, idx_ap, n, n, 128, queue_num=gq[0]
                )
                gq[0] = (gq[0] + 1) % 4
                if gchain[0] is not None:
                    bass._add_dep_helper(
                        ins.ins, gchain[0].ins, sync=False,
                        reason="pin SWDGE pool order for queue/lane pairing",
                    )
                gchain[0] = ins
                return ins

            def agg_win(w, tableA_ap, tableB_ap, scols, extra=()):
                """Gather + indicator matmuls for window w.

                Returns the PSUM tile with [scols, 128] = transposed,
                deg_inv-normalized aggregation. `extra` = [(stationary,
                moving), ...] appended to the same accumulation group.
                """
                slabs = []
                for half, gix, tab in ((0, gixA, tableA_ap), (1, gixB, tableB_ap)):
                    slab = slabp.tile([128, NCH, 128], BF16, tag=f"slab{half}")
                    base = w * (CAP // 16)
                    lo = 0
                    for n in SPLITS[w % 2]:
                        gather_chained(
                            slab[:, lo // 128:(lo + n) // 128, :], tab,
                            gix[:, base + lo // 16:base + (lo + n) // 16],
                            n,
                        )
                        lo += n
                    slabs.append(slab)
                ind = indp.tile([128, 2 * NCH, 128], BF16, tag="ind")
                nc.vector.tensor_tensor(
                    ind[:],
                    _bcast3(drt[:, w * 2 * NCH:(w + 1) * 2 * NCH],
                            2 * NCH, 128, "inner0"),
                    _bcast3(wt["iota"][:], 2 * NCH, 128, "mid0"),
                    mybir.AluOpType.is_equal,
                )
                nc.vector.tensor_tensor(
                    ind[:], ind[:],
                    _bcast3(wvt[:, w * 2 * NCH:(w + 1) * 2 * NCH],
                            2 * NCH, 128, "inner0"),
                    mybir.AluOpType.mult,
                )
                pagg = paggp.tile([128, 128], F32, tag="pagg")
                n_mm = 2 * NCH + len(extra)
                k = 0
                for half in (0, 1):
                    for c in range(NCH):
                        nc.tensor.matmul(
                            pagg[0:scols, :], slabs[half][:, c, 0:scols],
                            ind[:, half * NCH + c, :],
                            start=(k == 0), stop=(k == n_mm - 1),
                        )
                        k += 1
                for st, mv in extra:
                    nc.tensor.matmul(pagg[:], st, mv,
                                     start=False, stop=(k == n_mm - 1))
                    k += 1
                return pagg

            def fire_ag(loc, full):
                nc.gpsimd.collective_compute(
                    "AllGather", mybir.AluOpType.bypass, replica_groups=RG,
                    ins=[loc[:]], outs=[full[:]],
                )

            # ======================= L1 =======================
            for w in range(NW):
                ws = slice(w * 128, (w + 1) * 128)
                pagg = agg_win(w, x_pad[0:HALF_A, :], x_pad[HALF_A:RT, :], 64)
                a1 = sm.tile([64, 128], BF16, tag="a1")
                nc.scalar.activation(a1[:], pagg[0:64, :], COPY)
                for half, h1t in ((0, h1T0), (1, h1T1)):
                    hs = slice(half * 128, (half + 1) * 128)
                    ph = php.tile([128, 128], F32, tag="ph")
                    nc.tensor.matmul(ph[:], wt["W1_rel"][:, hs], a1[:],
                                     start=True, stop=False)
                    nc.tensor.matmul(ph[:], wt["W1_root"][:, hs], x_tr[:, ws],
                                     start=False, stop=False)
                    nc.tensor.matmul(ph[:], wt["b1"][:, hs], onesrow[:],
                                     start=False, stop=True)
                    nc.scalar.activation(h1t[:, ws], ph[:], RELU)
                # m2 = h1 @ W2_rel (node-major rows for the gather table)
                pm = php.tile([128, 128], F32, tag="ph")
                nc.tensor.matmul(pm[:], h1T0[:, ws], wt["W2_rel_h0"][:],
                                 start=True, stop=False)
                nc.tensor.matmul(pm[:], h1T1[:, ws], wt["W2_rel_h1"][:],
                                 start=False, stop=True)
                stg = stgp.tile([128, D2], BF16, tag="m2stg")
                nc.scalar.activation(stg[:], pm[:], COPY)
                nc.sync.dma_start(m2loc[w * 128:(w + 1) * 128, :], stg[:])
            fire_ag(m2loc, m2_full)

            # ======================= L2 =======================
            for w in range(NW if phases >= 2 else 0):
                ws = slice(w * 128, (w + 1) * 128)
                pagg = agg_win(
                    w, m2_full[0:HALF_A, :], m2_full[HALF_A:RT, :], D2,
                    extra=[
                        (wt["W2_root_h0"][:], h1T0[:, ws]),
                        (wt["W2_root_h1"][:], h1T1[:, ws]),
                        (wt["b2"][:], onesrow[:]),
                    ],
                )
                nc.scalar.activation(h2T[:, ws], pagg[:], RELU)
                ptr = ptrp.tile([128, 128], BF16, tag="ptr")
                nc.tensor.transpose(ptr[:], h2T[:, ws], ident[:])
                stg = stgp.tile([128, D2], BF16, tag="m2stg")
                nc.scalar.activation(stg[:], ptr[:], COPY)
                nc.sync.dma_start(h2loc[w * 128:(w + 1) * 128, :], stg[:])
            if phases >= 2:
                fire_ag(h2loc, h2_full)

            # ======================= heads =======================
            for w in range(NW if phases >= 3 else 0):
                ws = slice(w * 128, (w + 1) * 128)
                pagg = agg_win(w, h2_full[0:HALF_A, :], h2_full[HALF_A:RT, :],
                               D2)
                a3 = sm.tile([128, 128], BF16, tag="a3")
                nc.scalar.activation(a3[:], pagg[:], COPY)
                po = php.tile([128, 128], F32, tag="ph")
                nc.tensor.matmul(po[0:2 * OC, :], wt["Wheads"][:], a3[:],
                                 start=True, stop=False)
                nc.tensor.matmul(po[0:2 * OC, :], wt["Wroots"][:], h2T[:, ws],
                                 start=False, stop=False)
                nc.tensor.matmul(po[0:2 * OC, :], wt["bheads"][:], onesrow[:],
                                 start=False, stop=True)
                ostg = stgp.tile([2 * OC, 128], F32, tag="ostg")
                nc.scalar.activation(ostg[:], po[0:2 * OC, :], COPY)
                nc.sync.dma_start(out_d[:, ws], ostg[:])

            if phases < 3:
                dbg = {1: h1T0, 2: h2T}[phases]
                for w in range(NW):
                    ws = slice(w * 128, (w + 1) * 128)
                    stg = stgp.tile([2 * OC, 128], F32, tag="ostg")
                    nc.vector.tensor_copy(stg[:], dbg[0:2 * OC, ws])
                    nc.sync.dma_start(out_d[:, ws], stg[:])

    nc.compile()
    return nc


def get_nc():
    global _NC_CACHE
    if _NC_CACHE is None:
        _NC_CACHE = _build_nc()
    return _NC_CACHE


# ---------------------------------------------------------------- host prep

def _row_of(node: np.ndarray) -> np.ndarray:
    """Node id -> row in the shared (core, local) table layout."""
    o = node // LOCAL
    return o * LOCAL_PAD + (node - o * LOCAL)


def _wrap_idx16(vals: np.ndarray, nslots: int) -> np.ndarray:
    a = vals.astype(np.int16).reshape(nslots // 16, 16).T  # [16, W]
    return np.tile(a, (8, 1))


def _prep_core(src_row, dst_local, wv_edge, core_mask):
    """Build gidxA/B, drel, wval arrays for one core.

    Slots within each (window, half) are ordered by ascending table row so
    consecutive gather descriptors have ascending HBM addresses.
    drel/wval use the combined-halves layout: col = w*2*NCH + half*NCH + c.
    """
    row = src_row[core_mask]
    dl = dst_local[core_mask]
    wv = wv_edge[core_mask]
    win = (dl >> 7).astype(np.int64)
    rel = (dl & 127).astype(np.float32)

    out = {}
    dvals = np.full((128, NW * 2 * NCH), -1.0, np.float32)
    wvals = np.zeros((128, NW * 2 * NCH), np.float32)
    for half in (0, 1):
        sel = (row < HALF_A) if half == 0 else (row >= HALF_A)
        r = row[sel] - (0 if half == 0 else HALF_A)
        wvv = win[sel]
        rv = rel[sel]
        ww = wv[sel]
        order = np.lexsort((r, wvv))
        r, wvv, rv, ww = r[order], wvv[order], rv[order], ww[order]
        counts = np.bincount(wvv, minlength=NW)
        if counts.max() > CAP:
            raise RuntimeError(f"window overflow: {counts.max()} > {CAP}")
        starts = np.zeros(NW, np.int64)
        starts[1:] = np.cumsum(counts)[:-1]
        pos = np.arange(len(wvv)) - np.repeat(starts, counts)
        slot = wvv * CAP + pos
        nslots = NW * CAP
        gvals = np.zeros(nslots, np.int64)
        gvals[slot] = r
        out["gidx" + ("A" if half == 0 else "B")] = _wrap_idx16(gvals, nslots)
        cols = wvv * (2 * NCH) + half * NCH + pos // 128
        dvals[pos % 128, cols] = rv
        wvals[pos % 128, cols] = ww
    out["drel"] = dvals.astype(BF)
    out["wval"] = wvals.astype(BF)
    return out


def _pack_weights(i):
    wb = np.zeros(WB_LEN, np.float32)

    def put(name, arr):
        off, r, c = _WB_OFF[name]
        wb[off:off + r * c] = np.asarray(arr, np.float32).reshape(r * c)

    put("iota", np.tile(np.arange(128, dtype=np.float32), (128, 1)))
    put("ident", np.eye(128, dtype=np.float32))
    put("W1_rel", i["W1_rel"])
    put("W1_root", i["W1_root"])
    put("b1", i["b1"][None, :])
    put("W2_rel_h0", i["W2_rel"][0:128])
    put("W2_rel_h1", i["W2_rel"][128:256])
    put("W2_root_h0", i["W2_root"][0:128])
    put("W2_root_h1", i["W2_root"][128:256])
    put("b2", i["b2"][None, :])
    put("Wheads", np.concatenate([i["Wmu_rel"], i["Wls_rel"]], 1))
    put("Wroots", np.concatenate([i["Wmu_root"], i["Wls_root"]], 1))
    put("bheads", np.concatenate([i["bmu"], i["bls"]])[None, :])
    return wb.astype(BF)


def kernel(**inputs):
    x = np.asarray(inputs["x"], np.float32)
    ei = np.asarray(inputs["edge_index"])
    src = ei[0].astype(np.int64)
    dst = ei[1].astype(np.int64)

    deg = np.bincount(dst, minlength=N_NODES).astype(np.float32)
    dinv = np.where(deg > 0, 1.0 / np.maximum(deg, 1.0), 0.0).astype(np.float32)

    owner = dst // LOCAL
    dst_local = dst - owner * LOCAL
    src_row = _row_of(src)
    wv_edge = dinv[dst]

    x_pad = np.zeros((RT, 128), BF)
    rows_all = _row_of(np.arange(N_NODES))
    x_pad[rows_all, 0:64] = x.astype(BF)

    wb = _pack_weights({k: np.asarray(v, np.float32) for k, v in inputs.items()
                        if k not in ("x", "edge_index")})

    in_maps = []
    for c in range(N_CORES):
        m = _prep_core(src_row, dst_local, wv_edge, owner == c)
        x_tr = np.zeros((64, LOCAL_PAD), BF)
        x_tr[:, :LOCAL] = x[c * LOCAL:(c + 1) * LOCAL].T.astype(BF)
        m["x_pad"] = x_pad
        m["x_tr"] = x_tr
        m["wb"] = wb
        in_maps.append(m)

    from concourse.bass_utils import run_bass_kernel_spmd

    nc = get_nc()
    res = run_bass_kernel_spmd(nc, in_maps, list(range(N_CORES)))

    mu = np.zeros((N_NODES, OC), np.float32)
    ls = np.zeros((N_NODES, OC), np.float32)
    for c in range(N_CORES):
        o = np.asarray(res.results[c]["out"], np.float32)[:, :LOCAL].T
        mu[c * LOCAL:(c + 1) * LOCAL] = o[:, :OC]
        ls[c * LOCAL:(c + 1) * LOCAL] = o[:, OC:]
    return (mu, ls)


# revision 3
# speedup vs baseline: 1.0212x; 1.0044x over previous
"""GNN message-passing encoder (PyG GraphConv x4) on 8 TRN2 NeuronCores, v2.

Strategy (graph/data parallel): nodes sharded by dst range; per layer the
aggregation segment_sum(x[src], dst) runs as dma_gather of a replicated DRAM
table + per-window indicator matmuls accumulated in PSUM.

v2 design notes:
  - bf16 tables + bf16 indicator matmuls (fp32 PSUM accumulate).
  - indicators (ind[s,d] = dinv[dst_s] * (drel_s==d)) are built on the DVE
    (is_equal vs iota, then mult by the per-slot deg_inv), both halves in one
    [128, 18, 128] tile -> 2 Vector ops per window. The Vector queue carries
    ONLY these ops (all copies ride Scalar), so it never serializes the
    window pipeline. (DMA-ing host-precomputed indicators was tried: the
    640KB/window stream saturated the DMA engines and slowed the gathers.)
  - deg_inv is folded into the indicator values, so the aggregation comes out
    pre-normalized and TRANSPOSED ([ch, dst]) via slab-stationary matmuls.
  - one shared table row layout for all 3 phases (core-major), so one gather
    index set serves x, m2 and h2.
  - gathers: CAP=1152 per (window,half), calls of [640,512] per half with the
    order alternating by window parity -> all 4 SWDGE queues carry equal
    descriptor load (>1024 per call hangs the ucode; unbalanced queues halve
    the drain rate). Slots sorted by table row within each region.
  - all PSUM->SBUF copies ride the Scalar engine (activation Copy), keeping
    the Vector queue out of the dependency chain entirely.
  - one serial AllGather per phase boundary: chunked in-loop AGs were tried
    (7x1.8MB and 3x5MB variants) and LOST - collective DMA contends with the
    gather descriptor stream for the 16 DMA engines, slowing windows ~10%
    and running the AGs themselves at 35-87GB/s vs ~200GB/s uncontended.
  - heads aggregate h2 directly (transform after aggregation), so phase 3
    shares the h2 table with the root term.
"""

import sys

sys.path.insert(0, "/opt/trn_rl_repo")

import numpy as np
import ml_dtypes

import concourse.bass as bass
import concourse.bacc as bacc
import concourse.mybir as mybir
from concourse import tile
from concourse.bass import AP

# ---------------------------------------------------------------- constants
IN_CH = 64
OC = 16
D1 = 256
D2 = 128
N_CORES = 8

N_NODES = 50000
N_EDGES = 800000
LOCAL = N_NODES // N_CORES          # 6250
NW = 49                             # dst windows of 128 per core
LOCAL_PAD = NW * 128                # 6272
RT = N_CORES * LOCAL_PAD            # 50176
HALF_A = 25088                      # rows [0,HALF_A) use gidxA (int16 range)
CAP = 1152                          # slots per (window, half); max real fill 1137
NCH = CAP // 128                    # 9 slab chunks per half
SPLITS = ([640, 512], [512, 640])   # per-window call sizes, alternating so all
                                    # 4 SWDGE queues carry equal load

F32 = mybir.dt.float32
BF16 = mybir.dt.bfloat16
I16 = mybir.dt.int16

BF = ml_dtypes.bfloat16

# weights blob layout: name -> (rows, cols), all bf16
_WB_SPEC = [
    ("iota", 128, 128),
    ("ident", 128, 128),
    ("W1_rel", 64, D1),
    ("W1_root", 64, D1),
    ("b1", 1, D1),
    ("W2_rel_h0", 128, D2),
    ("W2_rel_h1", 128, D2),
    ("W2_root_h0", 128, D2),
    ("W2_root_h1", 128, D2),
    ("b2", 1, D2),
    ("Wheads", D2, 2 * OC),
    ("Wroots", D2, 2 * OC),
    ("bheads", 1, 2 * OC),
]
_WB_OFF = {}
_off = 0
for _n, _r, _c in _WB_SPEC:
    _WB_OFF[_n] = (_off, _r, _c)
    _off += _r * _c
WB_LEN = _off

_NC_CACHE = None


def _bcast3(ap2d: AP, mid: int, inner: int, mode: str) -> AP:
    """[128, X] -> [128, mid, inner] with a 0-stride broadcast dim."""
    if mode == "inner0":   # [128, mid] -> [128, mid, inner], inner step 0
        return AP(ap2d.tensor, ap2d.offset, [ap2d.ap[0], ap2d.ap[1], [0, inner]])
    if mode == "mid0":     # [128, inner] -> [128, mid, inner], mid step 0
        return AP(ap2d.tensor, ap2d.offset, [ap2d.ap[0], [0, mid], ap2d.ap[1]])
    raise ValueError(mode)


def _build_nc():
    import os
    phases = int(os.environ.get("GNN2_PHASES", "3"))
    nc = bacc.Bacc(None, num_swdge_queues=4, dynamic_dma_scratch_size=32768)

    # ---- IO
    x_pad = nc.dram_tensor("x_pad", [RT, 128], BF16, kind="ExternalInput")
    x_tr_d = nc.dram_tensor("x_tr", [64, LOCAL_PAD], BF16, kind="ExternalInput")
    gidxA = nc.dram_tensor("gidxA", [128, NW * CAP // 16], I16, kind="ExternalInput")
    gidxB = nc.dram_tensor("gidxB", [128, NW * CAP // 16], I16, kind="ExternalInput")
    drel_d = nc.dram_tensor("drel", [128, NW * 2 * NCH], BF16, kind="ExternalInput")
    wval_d = nc.dram_tensor("wval", [128, NW * 2 * NCH], BF16, kind="ExternalInput")
    wb = nc.dram_tensor("wb", [WB_LEN], BF16, kind="ExternalInput")
    out_d = nc.dram_tensor("out", [2 * OC, LOCAL_PAD], F32, kind="ExternalOutput")

    # ---- internal DRAM
    m2loc = nc.dram_tensor("m2loc", [LOCAL_PAD, D2], BF16)
    h2loc = nc.dram_tensor("h2loc", [LOCAL_PAD, D2], BF16)
    m2_full = nc.dram_tensor("m2_full", [RT, D2], BF16, addr_space="Shared")
    h2_full = nc.dram_tensor("h2_full", [RT, D2], BF16, addr_space="Shared")
    warm_in = nc.dram_tensor("warm_in", [1, 128], BF16)
    warm_out = nc.dram_tensor("warm_out", [N_CORES, 128], BF16, addr_space="Shared")

    RG = [list(range(N_CORES))]

    with tile.TileContext(nc) as tc:
        with (
            tc.tile_pool(name="cst", bufs=1) as cst,
            tc.tile_pool(name="big", bufs=1) as big,
            tc.tile_pool(name="slab", bufs=3) as slabp,
            tc.tile_pool(name="ind", bufs=3) as indp,
            tc.tile_pool(name="sm", bufs=3) as sm,
            tc.tile_pool(name="stg", bufs=2) as stgp,
            tc.tile_pool(name="pagg", bufs=2, space="PSUM") as paggp,
            tc.tile_pool(name="ph", bufs=3, space="PSUM") as php,
            tc.tile_pool(name="ptr", bufs=2, space="PSUM") as ptrp,
        ):
            # ---------------- constants / weights
            wt = {}
            for name, r, c in _WB_SPEC:
                t = cst.tile([r, c], BF16, tag=f"w_{name}")
                off = _WB_OFF[name][0]
                nc.sync.dma_start(
                    t[:], wb[off:off + r * c].rearrange("(r c) -> r c", c=c)
                )
                wt[name] = t
            onesrow = cst.tile([1, 128], BF16, tag="onesrow")
            nc.vector.memset(onesrow[:], 1.0)

            gixA = cst.tile([128, NW * CAP // 16], I16, tag="gixA")
            gixB = cst.tile([128, NW * CAP // 16], I16, tag="gixB")
            drt = cst.tile([128, NW * 2 * NCH], BF16, tag="drt")
            wvt = cst.tile([128, NW * 2 * NCH], BF16, tag="wvt")
            nc.sync.dma_start(gixA[:], gidxA[:])
            nc.sync.dma_start(gixB[:], gidxB[:])
            nc.sync.dma_start(drt[:], drel_d[:])
            nc.sync.dma_start(wvt[:], wval_d[:])

            x_tr = big.tile([64, LOCAL_PAD], BF16, tag="x_tr")
            nc.sync.dma_start(x_tr[:], x_tr_d[:])

            # tiny warmup collective: absorbs the ~11us first-collective init
            # during the startup window loads instead of at the L1->L2 boundary
            nc.gpsimd.collective_compute(
                "AllGather", mybir.AluOpType.bypass, replica_groups=RG,
                ins=[warm_in[:]], outs=[warm_out[:]],
            )

            h1T0 = big.tile([128, LOCAL_PAD], BF16, tag="h1T0")
            h1T1 = big.tile([128, LOCAL_PAD], BF16, tag="h1T1")
            h2T = big.tile([128, LOCAL_PAD], BF16, tag="h2T")

            ident = wt["ident"]
            COPY = mybir.ActivationFunctionType.Copy
            RELU = mybir.ActivationFunctionType.Relu

            # SWDGE queue discipline: gathers chained with no-sync deps to pin
            # their POOL order to emission order; queues rotate with period 4
            # (8 DMASW sem lanes % 4 queues -> each lane sees one queue).
            gchain = [None]
            gq = [0]

            def gather_chained(out_ap, table_ap, idx_ap, n):
                ins = nc.gpsimd.dma_gather(
                    out_ap, table_ap, idx_ap, n, n, 128, queue_num=gq[0]
                )
                gq[0] = (gq[0] + 1) % 4
                if gchain[0] is not None:
                    bass._add_dep_helper(
                        ins.ins, gchain[0].ins, sync=False,
                        reason="pin SWDGE pool order for queue/lane pairing",
                    )
                gchain[0] = ins
                return ins

            def agg_win(w, tableA_ap, tableB_ap, scols, extra=()):
                """Gather + indicator matmuls for window w.

                Returns the PSUM tile with [scols, 128] = transposed,
                deg_inv-normalized aggregation. `extra` = [(stationary,
                moving), ...] appended to the same accumulation group.
                """
                slabs = []
                for half, gix, tab in ((0, gixA, tableA_ap), (1, gixB, tableB_ap)):
                    slab = slabp.tile([128, NCH, 128], BF16, tag=f"slab{half}")
                    base = w * (CAP // 16)
                    lo = 0
                    for n in SPLITS[w % 2]:
                        gather_chained(
                            slab[:, lo // 128:(lo + n) // 128, :], tab,
                            gix[:, base + lo // 16:base + (lo + n) // 16],
                            n,
                        )
                        lo += n
                    slabs.append(slab)
                ind = indp.tile([128, 2 * NCH, 128], BF16, tag="ind")
                nc.vector.tensor_tensor(
                    ind[:],
                    _bcast3(drt[:, w * 2 * NCH:(w + 1) * 2 * NCH],
                            2 * NCH, 128, "inner0"),
                    _bcast3(wt["iota"][:], 2 * NCH, 128, "mid0"),
                    mybir.AluOpType.is_equal,
                )
                nc.vector.tensor_tensor(
                    ind[:], ind[:],
                    _bcast3(wvt[:, w * 2 * NCH:(w + 1) * 2 * NCH],
                            2 * NCH, 128, "inner0"),
                    mybir.AluOpType.mult,
                )
                pagg = paggp.tile([128, 128], F32, tag="pagg")
                n_mm = 2 * NCH + len(extra)
                k = 0
                for half in (0, 1):
                    for c in range(NCH):
                        nc.tensor.matmul(
                            pagg[0:scols, :], slabs[half][:, c, 0:scols],
                            ind[:, half * NCH + c, :],
                            start=(k == 0), stop=(k == n_mm - 1),
                        )
                        k += 1
                for st, mv in extra:
                    nc.tensor.matmul(pagg[:], st, mv,
                                     start=False, stop=(k == n_mm - 1))
                    k += 1
                return pagg

            def fire_ag(loc, full):
                nc.gpsimd.collective_compute(
                    "AllGather", mybir.AluOpType.bypass, replica_groups=RG,
                    ins=[loc[:]], outs=[full[:]],
                )

            # ======================= L1 =======================
            for w in range(NW):
                ws = slice(w * 128, (w + 1) * 128)
                pagg = agg_win(w, x_pad[0:HALF_A, :], x_pad[HALF_A:RT, :], 64)
                a1 = sm.tile([64, 128], BF16, tag="a1")
                nc.scalar.activation(a1[:], pagg[0:64, :], COPY)
                for half, h1t in ((0, h1T0), (1, h1T1)):
                    hs = slice(half * 128, (half + 1) * 128)
                    ph = php.tile([128, 128], F32, tag="ph")
                    nc.tensor.matmul(ph[:], wt["W1_rel"][:, hs], a1[:],
                                     start=True, stop=False)
                    nc.tensor.matmul(ph[:], wt["W1_root"][:, hs], x_tr[:, ws],
                                     start=False, stop=False)
                    nc.tensor.matmul(ph[:], wt["b1"][:, hs], onesrow[:],
                                     start=False, stop=True)
                    nc.scalar.activation(h1t[:, ws], ph[:], RELU)
                # m2 = h1 @ W2_rel (node-major rows for the gather table)
                pm = php.tile([128, 128], F32, tag="ph")
                nc.tensor.matmul(pm[:], h1T0[:, ws], wt["W2_rel_h0"][:],
                                 start=True, stop=False)
                nc.tensor.matmul(pm[:], h1T1[:, ws], wt["W2_rel_h1"][:],
                                 start=False, stop=True)
                stg = stgp.tile([128, D2], BF16, tag="m2stg")
                nc.scalar.activation(stg[:], pm[:], COPY)
                nc.sync.dma_start(m2loc[w * 128:(w + 1) * 128, :], stg[:])
            fire_ag(m2loc, m2_full)

            # ======================= L2 =======================
            for w in range(NW if phases >= 2 else 0):
                ws = slice(w * 128, (w + 1) * 128)
                pagg = agg_win(
                    w, m2_full[0:HALF_A, :], m2_full[HALF_A:RT, :], D2,
                    extra=[
                        (wt["W2_root_h0"][:], h1T0[:, ws]),
                        (wt["W2_root_h1"][:], h1T1[:, ws]),
                        (wt["b2"][:], onesrow[:]),
                    ],
                )
                nc.scalar.activation(h2T[:, ws], pagg[:], RELU)
                ptr = ptrp.tile([128, 128], BF16, tag="ptr")
                nc.tensor.transpose(ptr[:], h2T[:, ws], ident[:])
                stg = stgp.tile([128, D2], BF16, tag="m2stg")
                nc.scalar.activation(stg[:], ptr[:], COPY)
                nc.sync.dma_start(h2loc[w * 128:(w + 1) * 128, :], stg[:])
            if phases >= 2:
                fire_ag(h2loc, h2_full)

            # ======================= heads =======================
            for w in range(NW if phases >= 3 else 0):
                ws = slice(w * 128, (w + 1) * 128)
                pagg = agg_win(w, h2_full[0:HALF_A, :], h2_full[HALF_A:RT, :],
                               D2)
                a3 = sm.tile([128, 128], BF16, tag="a3")
                nc.scalar.activation(a3[:], pagg[:], COPY)
                po = php.tile([128, 128], F32, tag="ph")
                nc.tensor.matmul(po[0:2 * OC, :], wt["Wheads"][:], a3[:],
                                 start=True, stop=False)
                nc.tensor.matmul(po[0:2 * OC, :], wt["Wroots"][:], h2T[:, ws],
                                 start=False, stop=False)
                nc.tensor.matmul(po[0:2 * OC, :], wt["bheads"][:], onesrow[:],
                                 start=False, stop=True)
                ostg = stgp.tile([2 * OC, 128], F32, tag="ostg")
                nc.scalar.activation(ostg[:], po[0:2 * OC, :], COPY)
                nc.sync.dma_start(out_d[:, ws], ostg[:])

            if phases < 3:
                dbg = {1: h1T0, 2: h2T}[phases]
                for w in range(NW):
                    ws = slice(w * 128, (w + 1) * 128)
                    stg = stgp.tile([2 * OC, 128], F32, tag="ostg")
                    nc.vector.tensor_copy(stg[:], dbg[0:2 * OC, ws])
                    nc.sync.dma_start(out_d[:, ws], stg[:])

    nc.compile()
    return nc


def get_nc():
    global _NC_CACHE
    if _NC_CACHE is None:
        _NC_CACHE = _build_nc()
    return _NC_CACHE


# ---------------------------------------------------------------- host prep

def _row_of(node: np.ndarray) -> np.ndarray:
    """Node id -> row in the shared (core, local) table layout."""
    o = node // LOCAL
    return o * LOCAL_PAD + (node - o * LOCAL)


def _wrap_idx16(vals: np.ndarray, nslots: int) -> np.ndarray:
    a = vals.astype(np.int16).reshape(nslots // 16, 16).T  # [16, W]
    return np.tile(a, (8, 1))


def _prep_core(src_row, dst_local, wv_edge, core_mask):
    """Build gidxA/B, drel, wval arrays for one core.

    Slots within each (window, half) are ordered by ascending table row so
    consecutive gather descriptors have ascending HBM addresses.
    drel/wval use the combined-halves layout: col = w*2*NCH + half*NCH + c.
    """
    row = src_row[core_mask]
    dl = dst_local[core_mask]
    wv = wv_edge[core_mask]
    win = (dl >> 7).astype(np.int64)
    rel = (dl & 127).astype(np.float32)

    out = {}
    dvals = np.full((128, NW * 2 * NCH), -1.0, np.float32)
    wvals = np.zeros((128, NW * 2 * NCH), np.float32)
    for half in (0, 1):
        sel = (row < HALF_A) if half == 0 else (row >= HALF_A)
        r = row[sel] - (0 if half == 0 else HALF_A)
        wvv = win[sel]
        rv = rel[sel]
        ww = wv[sel]
        order = np.lexsort((r, wvv))
        r, wvv, rv, ww = r[order], wvv[order], rv[order], ww[order]
        counts = np.bincount(wvv, minlength=NW)
        if counts.max() > CAP:
            raise RuntimeError(f"window overflow: {counts.max()} > {CAP}")
        starts = np.zeros(NW, np.int64)
        starts[1:] = np.cumsum(counts)[:-1]
        pos = np.arange(len(wvv)) - np.repeat(starts, counts)
        slot = wvv * CAP + pos
        nslots = NW * CAP
        gvals = np.zeros(nslots, np.int64)
        gvals[slot] = r
        out["gidx" + ("A" if half == 0 else "B")] = _wrap_idx16(gvals, nslots)
        cols = wvv * (2 * NCH) + half * NCH + pos // 128
        dvals[pos % 128, cols] = rv
        wvals[pos % 128, cols] = ww
    out["drel"] = dvals.astype(BF)
    out["wval"] = wvals.astype(BF)
    return out


def _pack_weights(i):
    wb = np.zeros(WB_LEN, np.float32)

    def put(name, arr):
        off, r, c = _WB_OFF[name]
        wb[off:off + r * c] = np.asarray(arr, np.float32).reshape(r * c)

    put("iota", np.tile(np.arange(128, dtype=np.float32), (128, 1)))
    put("ident", np.eye(128, dtype=np.float32))
    put("W1_rel", i["W1_rel"])
    put("W1_root", i["W1_root"])
    put("b1", i["b1"][None, :])
    put("W2_rel_h0", i["W2_rel"][0:128])
    put("W2_rel_h1", i["W2_rel"][128:256])
    put("W2_root_h0", i["W2_root"][0:128])
    put("W2_root_h1", i["W2_root"][128:256])
    put("b2", i["b2"][None, :])
    put("Wheads", np.concatenate([i["Wmu_rel"], i["Wls_rel"]], 1))
    put("Wroots", np.concatenate([i["Wmu_root"], i["Wls_root"]], 1))
    put("bheads", np.concatenate([i["bmu"], i["bls"]])[None, :])
    return wb.astype(BF)


def kernel(**inputs):
    x = np.asarray(inputs["x"], np.float32)
    ei = np.asarray(inputs["edge_index"])
    src = ei[0].astype(np.int64)
    dst = ei[1].astype(np.int64)

    deg = np.bincount(dst, minlength=N_NODES).astype(np.float32)
    dinv = np.where(deg > 0, 1.0 / np.maximum(deg, 1.0), 0.0).astype(np.float32)

    owner = dst // LOCAL
    dst_local = dst - owner * LOCAL
    src_row = _row_of(src)
    wv_edge = dinv[dst]

    x_pad = np.zeros((RT, 128), BF)
    rows_all = _row_of(np.arange(N_NODES))
    x_pad[rows_all, 0:64] = x.astype(BF)

    wb = _pack_weights({k: np.asarray(v, np.float32) for k, v in inputs.items()
                        if k not in ("x", "edge_index")})

    in_maps = []
    for c in range(N_CORES):
        m = _prep_core(src_row, dst_local, wv_edge, owner == c)
        x_tr = np.zeros((64, LOCAL_PAD), BF)
        x_tr[:, :LOCAL] = x[c * LOCAL:(c + 1) * LOCAL].T.astype(BF)
        m["x_pad"] = x_pad
        m["x_tr"] = x_tr
        m["wb"] = wb
        in_maps.append(m)

    from concourse.bass_utils import run_bass_kernel_spmd

    nc = get_nc()
    res = run_bass_kernel_spmd(nc, in_maps, list(range(N_CORES)))

    mu = np.zeros((N_NODES, OC), np.float32)
    ls = np.zeros((N_NODES, OC), np.float32)
    for c in range(N_CORES):
        o = np.asarray(res.results[c]["out"], np.float32)[:, :LOCAL].T
        mu[c * LOCAL:(c + 1) * LOCAL] = o[:, :OC]
        ls[c * LOCAL:(c + 1) * LOCAL] = o[:, OC:]
    return (mu, ls)
